# revision 22
# baseline (speedup 1.0000x reference)
"""v3: baseline schedule + surgical startup/tail fixes (fast weight DMAs
via host pre-arrangement, per-chunk x loads with chunk-paced first q/k
groups, ACT-assisted tail)."""

import ml_dtypes
import numpy as np

import concourse.bass as bass
import concourse.mybir as mybir
import concourse.tile as tile
from concourse import bacc
from concourse.bass_utils import run_bass_kernel_spmd

F32 = mybir.dt.float32
BF16 = mybir.dt.bfloat16
NPBF16 = ml_dtypes.bfloat16

E = 1024
NH = 16
DH = 64
NCORES = 8
HPC = NH // NCORES  # heads per core = 2
LF = HPC * DH  # local features per core = 128
NCHUNK = E // 128  # contraction chunks for the qkv projection = 8


def build_nc(B=2, S=2048):
    ST = 512  # q-tile width
    SH = S // 2  # s-half processed per xT load
    NST = SH // ST  # s-tiles per half = 2
    NTT = S // 128  # 128-row t-chunks per batch = 16
    NQ = S // ST  # q-tiles per batch = 4
    BS = B * S

    nc = bacc.Bacc("TRN2")
    xT = nc.dram_tensor("xT", [E, BS], BF16, kind="ExternalInput")
    wq = nc.dram_tensor("wq", [128, NCHUNK * LF], BF16, kind="ExternalInput")
    wk = nc.dram_tensor("wk", [128, NCHUNK * LF], BF16, kind="ExternalInput")
    wv = nc.dram_tensor("wv", [128, NCHUNK * LF], BF16, kind="ExternalInput")
    bq = nc.dram_tensor("bq", [LF, 1], F32, kind="ExternalInput")
    bk = nc.dram_tensor("bk", [LF, 1], F32, kind="ExternalInput")
    wp = nc.dram_tensor("wp", [LF, E], BF16, kind="ExternalInput")
    y = nc.dram_tensor("y", [BS, E], BF16, kind="ExternalOutput")

    mm = nc.tensor.matmul
    EXP_SCALE = (DH ** -0.5) / 256.0

    with tile.TileContext(nc) as tc:
        with (
            tc.tile_pool(name="consts", bufs=1) as consts,
            tc.tile_pool(name="xpool", bufs=24) as xpool,
            tc.tile_pool(name="acts", bufs=2) as acts,  # qT/kT bf16
            tc.tile_pool(name="vap", bufs=2) as vap,  # v2 [t,d] bf16
            tc.tile_pool(name="attp", bufs=7) as attp,  # a exp bf16
            tc.tile_pool(name="aop", bufs=3) as aop,  # per-qt aoT tiles
            tc.tile_pool(name="npool", bufs=3) as npool,
            tc.tile_pool(name="ypool", bufs=4) as ypool,
            tc.tile_pool(name="psA", bufs=2, space="PSUM") as psA,
            tc.tile_pool(name="psS", bufs=2, space="PSUM") as psS,
            tc.tile_pool(name="psO", bufs=2, space="PSUM") as psO,
        ):
            # ---- constants (DMAs emitted in ramp order below) ----
            wq_sb = consts.tile([128, NCHUNK, LF], BF16, tag="wq")
            wk_sb = consts.tile([128, NCHUNK, LF], BF16, tag="wk")
            wv_sb = consts.tile([128, NCHUNK, LF], BF16, tag="wv")
            wp_sb = consts.tile([LF, E], BF16, tag="wp")
            bq_sb = consts.tile([LF, 1], F32, tag="bq")
            bk_sb = consts.tile([LF, 1], F32, tag="bk")
            expb_sb = consts.tile([128, 1], F32, tag="expb")
            nc.vector.memset(expb_sb, -2.0)

            wq_r = wq.rearrange("p (c n) -> p c n", c=NCHUNK)
            wk_r = wk.rearrange("p (c n) -> p c n", c=NCHUNK)
            wv_r = wv.rearrange("p (c n) -> p c n", c=NCHUNK)
            xT_r = xT.rearrange("(c p) s -> p c s", p=128)

            # per-batch state; x is loaded in per-chunk tiles so matmuls can
            # start as soon as each 256KB slice lands
            qTs, kTs, v2s, aoTs, xts = {}, {}, {}, {}, {}

            def dma_x(b, sh, c):
                t = xpool.tile([128, 1, SH], BF16, tag="xt", name=f"x{b}{sh}{c}")
                s0 = b * S + sh * SH
                nc.sync.dma_start(out=t, in_=xT_r[:, c : c + 1, s0 : s0 + SH])
                xts[(b, sh, c)] = t

            def ensure_x(b, sh):
                if (b, sh, 0) not in xts:
                    for c in range(NCHUNK):
                        dma_x(b, sh, c)

            def emit_qk_group(b, sh, which):
                """One (s-half, q|k) block of the projection -> bf16."""
                if b not in qTs:
                    qTs[b] = acts.tile([128, S], BF16, tag="qT", name=f"qT{b}")
                    kTs[b] = acts.tile([128, S], BF16, tag="kT", name=f"kT{b}")
                dst, w_sb, b_sb = {
                    "q": (qTs[b], wq_sb, bq_sb),
                    "k": (kTs[b], wk_sb, bk_sb),
                }[which]
                ensure_x(b, sh)
                for st in range(NST):
                    ps = psA.tile([128, ST], F32, tag="psA")
                    for c in range(NCHUNK):
                        mm(
                            ps,
                            lhsT=w_sb[:, c, :],
                            rhs=xts[(b, sh, c)][:, 0, st * ST : (st + 1) * ST],
                            start=(c == 0),
                            stop=(c == NCHUNK - 1),
                        )
                    g0 = sh * SH + st * ST
                    # evac: psum (=16*x@w) + b~ -> bf16 q~ = 16*(q+b)
                    nc.vector.tensor_scalar_add(dst[:, g0 : g0 + ST], ps, b_sb)

            def emit_v_group(b, sh):
                """v~ = 16*v in [t, d] bf16 layout directly (v-direct)."""
                if b not in v2s:
                    v2s[b] = vap.tile(
                        [128, NTT, HPC, DH + 1], BF16, tag="v2", name=f"v2{b}"
                    )
                    # col DH = ones (denominator row)
                    nc.gpsimd.memset(v2s[b][:, :, :, DH : DH + 1], 1.0)
                v2 = v2s[b]
                ensure_x(b, sh)
                for sc in range(SH // 128):
                    scg = sh * (SH // 128) + sc  # global t-chunk id
                    psv = psA.tile([128, HPC, DH], F32, tag="psA")
                    for c in range(NCHUNK):
                        mm(
                            psv,
                            lhsT=xts[(b, sh, c)][:, 0, sc * 128 : (sc + 1) * 128],
                            rhs=wv_sb[:, c, :],
                            start=(c == 0),
                            stop=(c == NCHUNK - 1),
                        )
                    # evac psum (=16*v) -> v~ = 16*v in bf16
                    nc.vector.tensor_copy(v2[:, scg, :, 0:DH], psv)


            def emit_sc(b, qt, tt, a_tiles):
                """Scores + exp for one (q-tile, t-chunk), per head."""
                qT, kT = qTs[b], kTs[b]
                qsl = slice(qt * ST, (qt + 1) * ST)
                tsl = slice(tt * 128, (tt + 1) * 128)
                ps_s = psS.tile([128, HPC * ST], F32, tag="psS")
                for h in range(HPC):
                    hsl = slice(h * DH, (h + 1) * DH)
                    mm(
                        ps_s[:, h * ST : (h + 1) * ST],
                        lhsT=kT[hsl, tsl],
                        rhs=qT[hsl, qsl],
                        start=True,
                        stop=True,
                        tile_position=(h * DH, 0),
                    )
                a = attp.tile([128, HPC, ST], BF16, tag="a")
                a_tiles[tt] = a
                nc.scalar.activation(
                    a,
                    ps_s,
                    mybir.ActivationFunctionType.Exp,
                    bias=expb_sb,
                    scale=EXP_SCALE,
                )

            def emit_av(b, qt, tt, a_tiles, out_ps):
                """attn@V for one t-chunk; lazily allocates the psO tiles so
                their pool-reuse wait lands here, not on earlier scores."""
                v2 = v2s[b]
                if tt == 0:
                    for h in range(HPC):
                        out_ps.append(
                            psO.tile([128, ST], F32, tag="psO", name=f"psO_{h}")
                        )
                a = a_tiles[tt] if tt in a_tiles else a_tiles.pop(tt)
                for h in range(HPC):
                    mm(
                        out_ps[h][0 : DH + 1, :],
                        lhsT=v2[:, tt, h, :],
                        rhs=a[:, h, :],
                        start=(tt == 0),
                        stop=(tt == NTT - 1),
                    )
                a_tiles.pop(tt, None)

            def emit_norm_qt(b, qt, out_ps, tail=False):
                """Normalize this q-tile (denominator row DH of each psO).
                Writes a per-q-tile aoT tile so the projection of the
                previous q-tile has no false tile dependency on it.  At the
                kernel tail the first den copy rides the idle ACT engine."""
                aoT = aop.tile([128, ST], BF16, tag="aoT", name=f"ao{b}_{qt}")
                aoTs[(b, qt)] = aoT
                dens, recs, bcs = [], [], []
                for h in range(HPC):
                    den_sb = npool.tile([1, ST], F32, tag="den")
                    if tail and h == 0:
                        nc.scalar.activation(
                            den_sb,
                            out_ps[h][DH : DH + 1, :],
                            mybir.ActivationFunctionType.Copy,
                        )
                    else:
                        nc.vector.tensor_copy(den_sb, out_ps[h][DH : DH + 1, :])
                    dens.append(den_sb)
                for h in range(HPC):
                    rec = npool.tile([1, ST], F32, tag="rec")
                    nc.vector.reciprocal_approx_fast(rec, dens[h])
                    recs.append(rec)
                    bc_sb = npool.tile([DH, ST], F32, tag="bc")
                    nc.gpsimd.partition_broadcast(bc_sb, rec)
                    bcs.append(bc_sb)
                for h in range(HPC):
                    nc.vector.tensor_mul(
                        aoT[h * DH : (h + 1) * DH, :],
                        out_ps[h][0:DH, :],
                        bcs[h],
                    )

            def emit_proj_chunk(b, qt, st, eh, evac_act=False):
                """One (s128, e512) chunk of a q-tile's output projection."""
                aoT = aoTs[(b, qt)]
                s_loc = qt * ST + st * 128
                r0 = b * S + s_loc
                esl = slice(eh * 512, (eh + 1) * 512)
                ps_y = psA.tile([128, 512], F32, tag="psA")
                mm(
                    ps_y,
                    lhsT=aoT[:, st * 128 : (st + 1) * 128],
                    rhs=wp_sb[:, esl],
                    start=True,
                    stop=True,
                )
                y_sb = ypool.tile([128, 512], BF16, tag="y")
                if evac_act:
                    nc.scalar.activation(
                        y_sb, ps_y, mybir.ActivationFunctionType.Copy
                    )
                else:
                    nc.vector.tensor_copy(y_sb, ps_y)
                nc.sync.dma_start(out=y[r0 : r0 + 128, esl], in_=y_sb)

            # ---- emission schedule ----
            def emit_qk_pair(b, sh, st):
                """q AND k for one (half, st), interleaved per chunk so the
                matmuls track the per-chunk x DMAs during the ramp."""
                if b not in qTs:
                    qTs[b] = acts.tile([128, S], BF16, tag="qT", name=f"qT{b}")
                    kTs[b] = acts.tile([128, S], BF16, tag="kT", name=f"kT{b}")
                psq = psA.tile([128, ST], F32, tag="psA", name=f"pq{b}{sh}{st}")
                psk = psA.tile([128, ST], F32, tag="psA", name=f"pk{b}{sh}{st}")
                csl = slice(st * ST, (st + 1) * ST)
                for c in range(NCHUNK):
                    xc = xts[(b, sh, c)][:, 0, csl]
                    mm(psq, lhsT=wq_sb[:, c, :], rhs=xc,
                       start=(c == 0), stop=(c == NCHUNK - 1),
                       skip_group_check=True)
                    mm(psk, lhsT=wk_sb[:, c, :], rhs=xc,
                       start=(c == 0), stop=(c == NCHUNK - 1),
                       skip_group_check=True)
                g0 = sh * SH + st * ST
                nc.vector.tensor_scalar_add(qTs[b][:, g0 : g0 + ST], psq, bq_sb)
                nc.vector.tensor_scalar_add(kTs[b][:, g0 : g0 + ST], psk, bk_sb)

            # DMA ring order: wq first, then x(b0,sh0) chunks (the critical
            # path), the other consts slotted where their consumers are
            nc.sync.dma_start(out=wq_sb, in_=wq_r[:, :, :])
            for c in range(4):
                dma_x(0, 0, c)
            nc.sync.dma_start(out=bq_sb, in_=bq[:, :])
            nc.sync.dma_start(out=wk_sb, in_=wk_r[:, :, :])
            for c in range(4, NCHUNK):
                dma_x(0, 0, c)
            nc.sync.dma_start(out=bk_sb, in_=bk[:, :])
            nc.sync.dma_start(out=wv_sb, in_=wv_r[:, :, :])
            for c in range(NCHUNK):
                dma_x(0, 1, c)
            nc.sync.dma_start(out=wp_sb, in_=wp[:, :])

            emit_qk_pair(0, 0, 0)
            emit_qk_pair(0, 0, 1)
            emit_v_group(0, 0)
            emit_qk_pair(0, 1, 0)
            emit_qk_pair(0, 1, 1)
            emit_v_group(0, 1)
            # interleave batch 1's A-phase into batch 0's attention
            items = [
                ("qk", 0, "q"), ("v", 0, None), ("qk", 0, "k"),
                ("qk", 1, "q"), ("v", 1, None), ("qk", 1, "k"),
            ]
            per_qt = -(-len(items) // NQ)
            interleave = {
                qt: items[qt * per_qt : (qt + 1) * per_qt] for qt in range(NQ)
            }
            # Global software pipeline: flat (b, qt, tt) stream; attn@V
            # trails the scores/exp by DELAY slots across q-tile and batch
            # boundaries; the previous q-tile's projection chunks are spread
            # one per slot so their psum evacuations never bunch up.
            DELAY = 5
            seq = [
                (b, qt, tt)
                for b in range(B)
                for qt in range(NQ)
                for tt in range(NTT)
            ]
            a_tiles = {}
            qt_state = {}  # (b, qt) -> out_ps list
            from collections import deque
            pending_proj = deque()

            def boundary(bq, qq, g):
                """av of (bq, qq) just completed: norm now, queue proj
                (first chunk held back 3 slots so it lands after the
                normalization chain has finished)."""
                emit_norm_qt(bq, qq, qt_state.pop((bq, qq)),
                             tail=(bq == B - 1 and qq == NQ - 1))
                for st in range(ST // 128):
                    for eh in range(E // 512):
                        pending_proj.append((g + 3, (bq, qq, st, eh)))
                if bq + 1 < B:
                    for item in interleave.get(qq, []):
                        kind, sh, which = item
                        if kind == "qk":
                            emit_qk_group(bq + 1, sh, which)
                        else:
                            emit_v_group(bq + 1, sh)

            for g, (b, qt, tt) in enumerate(seq):
                emit_sc(b, qt, tt, a_tiles)
                if g >= DELAY:
                    bb, qb, tb = seq[g - DELAY]
                    ops = qt_state.setdefault((bb, qb), [])
                    emit_av(bb, qb, tb, a_tiles, ops)
                    if tb == NTT - 1:
                        boundary(bb, qb, g)
                if pending_proj and pending_proj[0][0] <= g:
                    emit_proj_chunk(*pending_proj.popleft()[1])
            for g in range(len(seq) - DELAY, len(seq)):
                bb, qb, tb = seq[g]
                ops = qt_state.setdefault((bb, qb), [])
                emit_av(bb, qb, tb, a_tiles, ops)
                if tb == NTT - 1:
                    boundary(bb, qb, g)
            k = 0
            while pending_proj:
                emit_proj_chunk(*pending_proj.popleft()[1],
                                evac_act=(k % 2 == 1))
                k += 1

    nc.compile()
    return nc


_NC_CACHE = {}


def _get_nc(B, S):
    key = (B, S)
    if key not in _NC_CACHE:
        _NC_CACHE[key] = build_nc(B, S)
    return _NC_CACHE[key]


def _prearrange_w(w):
    """[E, LF] -> [128, NCHUNK*LF] with out[p, c*LF+n] = w[c*128+p, n]."""
    w3 = w.reshape(NCHUNK, 128, LF)  # [c, p, n]
    return np.ascontiguousarray(w3.transpose(1, 0, 2).reshape(128, NCHUNK * LF))


def make_in_maps(x, w_qkv, b_qkv, w_proj):
    B, S, _ = x.shape
    xT = np.ascontiguousarray(x.reshape(B * S, E).T).astype(NPBF16)
    in_maps = []
    for c in range(NCORES):
        cols = slice(c * LF, (c + 1) * LF)
        in_maps.append(
            {
                "xT": xT,
                "wq": _prearrange_w(
                    w_qkv[:, 0 * E : 1 * E][:, cols] * 16.0
                ).astype(NPBF16),
                "wk": _prearrange_w(
                    w_qkv[:, 1 * E : 2 * E][:, cols] * 16.0
                ).astype(NPBF16),
                "wv": _prearrange_w(
                    w_qkv[:, 2 * E : 3 * E][:, cols] * 16.0
                ).astype(NPBF16),
                "bq": (b_qkv[0 * E : 1 * E][cols] * 16.0)
                .reshape(LF, 1)
                .astype(np.float32),
                "bk": (b_qkv[1 * E : 2 * E][cols] * 16.0)
                .reshape(LF, 1)
                .astype(np.float32),
                "wp": np.ascontiguousarray(w_proj[cols, :] / 16.0).astype(
                    NPBF16
                ),
            }
        )
    return in_maps


def kernel_run(x, w_qkv, b_qkv, w_proj, b_proj, trace=False):
    x = np.asarray(x, dtype=np.float32)
    w_qkv = np.asarray(w_qkv, dtype=np.float32)
    b_qkv = np.asarray(b_qkv, dtype=np.float32)
    w_proj = np.asarray(w_proj, dtype=np.float32)
    b_proj = np.asarray(b_proj, dtype=np.float32)
    B, S, _ = x.shape
    nc = _get_nc(B, S)
    in_maps = make_in_maps(x, w_qkv, b_qkv, w_proj)
    res = run_bass_kernel_spmd(
        nc, in_maps, core_ids=list(range(NCORES)), trace=trace
    )
    y = res.results[0]["y"].astype(np.float64)
    for c in range(1, NCORES):
        y += res.results[c]["y"].astype(np.float64)
    # v-bias contribution: (b_v @ w_proj) constant row + b_proj
    bv = b_qkv[2 * E : 3 * E]
    y += (bv @ w_proj + b_proj)[None, :]
    return y.astype(np.float32).reshape(B, S, E), res


def kernel(x, w_qkv, b_qkv, w_proj, b_proj):
    y, _ = kernel_run(x, w_qkv, b_qkv, w_proj, b_proj)
    return y


# revision 25
# speedup vs baseline: 1.0110x; 1.0110x over previous
"""Multi-head attention (B=2, S=2048, E=1024, H=16) on 8 trn2 NeuronCores.

Sharding: tensor-parallel over heads (2 heads per core).  Each core computes
q/k/v for its 2 heads from the full x, runs attention, and produces a partial
output projection (row-split w_proj); the host sums the 8 partials and adds
b_proj plus the constant b_v @ w_proj row.

Schedule: software-pipelined (b, qt, tt) stream -- scores (2 heads packed
into the PE via tile_position K=64 row groups) -> exp on ACT -> attn@V
trailing by 5 slots with the softmax denominator riding as a 65th v-column;
per-q-tile normalization (DVE reciprocal + gpsimd partition broadcast);
batch 1's qkv projection interleaved into batch 0's attention at q-tile
boundaries; projections spread one chunk per slot.

Refinements over the 223.6us baseline (measured 217.2-218.7us):
  - weights host-pre-arranged to contiguous-per-partition layout (2KB
    descriptors, ~5x faster weight DMAs),
  - x streamed in per-contraction-chunk DMAs with the first q/k groups
    interleaved per chunk, so the PE starts ~7us earlier,
  - kernel tail: the last q-tile's norm den-copy and half its projection
    evacuations ride the then-idle ACT engine,
  - ~3us HAM-warmup spin (7 throwaway matmuls on a zeroed tile) during the
    initial DMA wait so the A-phase starts at the warm 2.4GHz clock, and
    batch 1's x chunks prefetched in the ramp (idle ring window) instead of
    lazily mid-attention where they contend with y writebacks.

NOTE (power wall): a fully gap-free rewrite (ACT-paced slot machine with
credit-scheduled filler) measured SLOWER (244-265us) -- sustaining ~99% PE
duty trips a chip-level power/DVFS throttle that drops every engine clock
11-21% from the first instruction (verified: identical instruction
encodings, uniform slowdown, version-locked A/B against this kernel).
This schedule stays at ~82% PE duty and keeps full clocks.  Also measured
and rejected on hardware: dual-DGE-ring ramp DMAs + boundary-covering proj
reserve (v7: 217.0/220.7/219.0us -- noise-equivalent), PE-mode-transpose
v-projection (v8: 226us, transpose latency exceeds the LDWEIGHTS saving),
gpsimd-ring x loads (v5: 222us, SWDGE descriptor generation is slow),
chunk-major host x layout with 8-16KB-descriptor mega/quarter DMAs (v9:
224us, v10: 224us -- larger descriptors do NOT raise the ~180GB/s
effective transfer rate, and the coarser tile dependencies delay the
first matmul to 19-21us vs 14us with per-chunk 256KB loads), bf16 y
writeback (v11: timing-identical, rel err 0.0070 vs 0.0061), and
w-stationary v-projection with DMA-XBAR transposes (v12: 255us --
correct, but the DMATranspose<->DMACopy xbar-mode serialization that
Tile inserts against the known HW hang drains the ring at every
transition, dwarfing the ~20us LDWEIGHTS saving).
Engine floors: PE ~180us busy (137us of moving columns + LDW/dispatch
overhead), ACT ~138us of exps; at the ~82% duty the power wall allows,
~217us is the observed equilibrium.

Scaling: q~ = 16*(q+b) bf16 (w_qkv*16 on host), scores psum = 256*score,
exp scale (1/sqrt(64))/256 with bias -2, v~ = 16*v, wp~ = w_proj/16.
Measured (v13): 217194/218473 ns bench vs contemporaneous v3 control at
218737 ns; v3 full-harness 217167-220122 ns (baseline 223573 ns), rel err
0.0061.
"""

import ml_dtypes
import numpy as np

import concourse.bass as bass
import concourse.mybir as mybir
import concourse.tile as tile
from concourse import bacc
from concourse.bass_utils import run_bass_kernel_spmd

F32 = mybir.dt.float32
BF16 = mybir.dt.bfloat16
NPBF16 = ml_dtypes.bfloat16

E = 1024
NH = 16
DH = 64
NCORES = 8
HPC = NH // NCORES  # heads per core = 2
LF = HPC * DH  # local features per core = 128
NCHUNK = E // 128  # contraction chunks for the qkv projection = 8


def build_nc(B=2, S=2048):
    ST = 512  # q-tile width
    SH = S // 2  # s-half processed per xT load
    NST = SH // ST  # s-tiles per half = 2
    NTT = S // 128  # 128-row t-chunks per batch = 16
    NQ = S // ST  # q-tiles per batch = 4
    BS = B * S

    nc = bacc.Bacc("TRN2")
    xT = nc.dram_tensor("xT", [E, BS], BF16, kind="ExternalInput")
    wq = nc.dram_tensor("wq", [128, NCHUNK * LF], BF16, kind="ExternalInput")
    wk = nc.dram_tensor("wk", [128, NCHUNK * LF], BF16, kind="ExternalInput")
    wv = nc.dram_tensor("wv", [128, NCHUNK * LF], BF16, kind="ExternalInput")
    bq = nc.dram_tensor("bq", [LF, 1], F32, kind="ExternalInput")
    bk = nc.dram_tensor("bk", [LF, 1], F32, kind="ExternalInput")
    wp = nc.dram_tensor("wp", [LF, E], BF16, kind="ExternalInput")
    y = nc.dram_tensor("y", [BS, E], F32, kind="ExternalOutput")

    mm = nc.tensor.matmul
    EXP_SCALE = (DH ** -0.5) / 256.0

    with tile.TileContext(nc) as tc:
        with (
            tc.tile_pool(name="consts", bufs=1) as consts,
            tc.tile_pool(name="xpool", bufs=32) as xpool,
            tc.tile_pool(name="acts", bufs=2) as acts,  # qT/kT bf16
            tc.tile_pool(name="vap", bufs=2) as vap,  # v2 [t,d] bf16
            tc.tile_pool(name="attp", bufs=7) as attp,  # a exp bf16
            tc.tile_pool(name="aop", bufs=3) as aop,  # per-qt aoT tiles
            tc.tile_pool(name="npool", bufs=3) as npool,
            tc.tile_pool(name="ypool", bufs=4) as ypool,
            tc.tile_pool(name="psA", bufs=2, space="PSUM") as psA,
            tc.tile_pool(name="psS", bufs=2, space="PSUM") as psS,
            tc.tile_pool(name="psO", bufs=2, space="PSUM") as psO,
        ):
            # ---- constants (DMAs emitted in ramp order below) ----
            wq_sb = consts.tile([128, NCHUNK, LF], BF16, tag="wq")
            wk_sb = consts.tile([128, NCHUNK, LF], BF16, tag="wk")
            wv_sb = consts.tile([128, NCHUNK, LF], BF16, tag="wv")
            wp_sb = consts.tile([LF, E], BF16, tag="wp")
            bq_sb = consts.tile([LF, 1], F32, tag="bq")
            bk_sb = consts.tile([LF, 1], F32, tag="bk")
            expb_sb = consts.tile([128, 1], F32, tag="expb")
            warm_sb = consts.tile([128, 512], BF16, tag="warm")
            nc.vector.memset(expb_sb, -2.0)
            nc.vector.memset(warm_sb, 0.0)

            wq_r = wq.rearrange("p (c n) -> p c n", c=NCHUNK)
            wk_r = wk.rearrange("p (c n) -> p c n", c=NCHUNK)
            wv_r = wv.rearrange("p (c n) -> p c n", c=NCHUNK)
            xT_r = xT.rearrange("(c p) s -> p c s", p=128)

            # per-batch state; x is loaded in per-chunk tiles so matmuls can
            # start as soon as each 256KB slice lands
            qTs, kTs, v2s, aoTs, xts = {}, {}, {}, {}, {}

            def dma_x(b, sh, c):
                t = xpool.tile([128, 1, SH], BF16, tag="xt", name=f"x{b}{sh}{c}")
                s0 = b * S + sh * SH
                nc.sync.dma_start(out=t, in_=xT_r[:, c : c + 1, s0 : s0 + SH])
                xts[(b, sh, c)] = t

            def ensure_x(b, sh):
                if (b, sh, 0) not in xts:
                    for c in range(NCHUNK):
                        dma_x(b, sh, c)

            def emit_qk_group(b, sh, which):
                """One (s-half, q|k) block of the projection -> bf16."""
                if b not in qTs:
                    qTs[b] = acts.tile([128, S], BF16, tag="qT", name=f"qT{b}")
                    kTs[b] = acts.tile([128, S], BF16, tag="kT", name=f"kT{b}")
                dst, w_sb, b_sb = {
                    "q": (qTs[b], wq_sb, bq_sb),
                    "k": (kTs[b], wk_sb, bk_sb),
                }[which]
                ensure_x(b, sh)
                for st in range(NST):
                    ps = psA.tile([128, ST], F32, tag="psA")
                    for c in range(NCHUNK):
                        mm(
                            ps,
                            lhsT=w_sb[:, c, :],
                            rhs=xts[(b, sh, c)][:, 0, st * ST : (st + 1) * ST],
                            start=(c == 0),
                            stop=(c == NCHUNK - 1),
                        )
                    g0 = sh * SH + st * ST
                    # evac: psum (=16*x@w) + b~ -> bf16 q~ = 16*(q+b)
                    nc.vector.tensor_scalar_add(dst[:, g0 : g0 + ST], ps, b_sb)

            def emit_v_group(b, sh):
                """v~ = 16*v in [t, d] bf16 layout directly (v-direct)."""
                if b not in v2s:
                    v2s[b] = vap.tile(
                        [128, NTT, HPC, DH + 1], BF16, tag="v2", name=f"v2{b}"
                    )
                    # col DH = ones (denominator row)
                    nc.gpsimd.memset(v2s[b][:, :, :, DH : DH + 1], 1.0)
                v2 = v2s[b]
                ensure_x(b, sh)
                for sc in range(SH // 128):
                    scg = sh * (SH // 128) + sc  # global t-chunk id
                    psv = psA.tile([128, HPC, DH], F32, tag="psA")
                    for c in range(NCHUNK):
                        mm(
                            psv,
                            lhsT=xts[(b, sh, c)][:, 0, sc * 128 : (sc + 1) * 128],
                            rhs=wv_sb[:, c, :],
                            start=(c == 0),
                            stop=(c == NCHUNK - 1),
                        )
                    # evac psum (=16*v) -> v~ = 16*v in bf16
                    nc.vector.tensor_copy(v2[:, scg, :, 0:DH], psv)


            def emit_sc(b, qt, tt, a_tiles):
                """Scores + exp for one (q-tile, t-chunk), per head."""
                qT, kT = qTs[b], kTs[b]
                qsl = slice(qt * ST, (qt + 1) * ST)
                tsl = slice(tt * 128, (tt + 1) * 128)
                ps_s = psS.tile([128, HPC * ST], F32, tag="psS")
                for h in range(HPC):
                    hsl = slice(h * DH, (h + 1) * DH)
                    mm(
                        ps_s[:, h * ST : (h + 1) * ST],
                        lhsT=kT[hsl, tsl],
                        rhs=qT[hsl, qsl],
                        start=True,
                        stop=True,
                        tile_position=(h * DH, 0),
                    )
                a = attp.tile([128, HPC, ST], BF16, tag="a")
                a_tiles[tt] = a
                nc.scalar.activation(
                    a,
                    ps_s,
                    mybir.ActivationFunctionType.Exp,
                    bias=expb_sb,
                    scale=EXP_SCALE,
                )

            def emit_av(b, qt, tt, a_tiles, out_ps):
                """attn@V for one t-chunk; lazily allocates the psO tiles so
                their pool-reuse wait lands here, not on earlier scores."""
                v2 = v2s[b]
                if tt == 0:
                    for h in range(HPC):
                        out_ps.append(
                            psO.tile([128, ST], F32, tag="psO", name=f"psO_{h}")
                        )
                a = a_tiles[tt] if tt in a_tiles else a_tiles.pop(tt)
                for h in range(HPC):
                    mm(
                        out_ps[h][0 : DH + 1, :],
                        lhsT=v2[:, tt, h, :],
                        rhs=a[:, h, :],
                        start=(tt == 0),
                        stop=(tt == NTT - 1),
                    )
                a_tiles.pop(tt, None)

            def emit_norm_qt(b, qt, out_ps, tail=False):
                """Normalize this q-tile (denominator row DH of each psO).
                Writes a per-q-tile aoT tile so the projection of the
                previous q-tile has no false tile dependency on it.  At the
                kernel tail the first den copy rides the idle ACT engine."""
                aoT = aop.tile([128, ST], BF16, tag="aoT", name=f"ao{b}_{qt}")
                aoTs[(b, qt)] = aoT
                dens, recs, bcs = [], [], []
                for h in range(HPC):
                    den_sb = npool.tile([1, ST], F32, tag="den")
                    if tail and h == 0:
                        nc.scalar.activation(
                            den_sb,
                            out_ps[h][DH : DH + 1, :],
                            mybir.ActivationFunctionType.Copy,
                        )
                    else:
                        nc.vector.tensor_copy(den_sb, out_ps[h][DH : DH + 1, :])
                    dens.append(den_sb)
                for h in range(HPC):
                    rec = npool.tile([1, ST], F32, tag="rec")
                    nc.vector.reciprocal_approx_fast(rec, dens[h])
                    recs.append(rec)
                    bc_sb = npool.tile([DH, ST], F32, tag="bc")
                    nc.gpsimd.partition_broadcast(bc_sb, rec)
                    bcs.append(bc_sb)
                for h in range(HPC):
                    nc.vector.tensor_mul(
                        aoT[h * DH : (h + 1) * DH, :],
                        out_ps[h][0:DH, :],
                        bcs[h],
                    )

            def emit_proj_chunk(b, qt, st, eh, evac_act=False):
                """One (s128, e512) chunk of a q-tile's output projection."""
                aoT = aoTs[(b, qt)]
                s_loc = qt * ST + st * 128
                r0 = b * S + s_loc
                esl = slice(eh * 512, (eh + 1) * 512)
                ps_y = psA.tile([128, 512], F32, tag="psA")
                mm(
                    ps_y,
                    lhsT=aoT[:, st * 128 : (st + 1) * 128],
                    rhs=wp_sb[:, esl],
                    start=True,
                    stop=True,
                )
                y_sb = ypool.tile([128, 512], F32, tag="y")
                if evac_act:
                    nc.scalar.activation(
                        y_sb, ps_y, mybir.ActivationFunctionType.Copy
                    )
                else:
                    nc.vector.tensor_copy(y_sb, ps_y)
                nc.sync.dma_start(out=y[r0 : r0 + 128, esl], in_=y_sb)

            # ---- emission schedule ----
            def emit_qk_pair(b, sh, st):
                """q AND k for one (half, st), interleaved per chunk so the
                matmuls track the per-chunk x DMAs during the ramp."""
                if b not in qTs:
                    qTs[b] = acts.tile([128, S], BF16, tag="qT", name=f"qT{b}")
                    kTs[b] = acts.tile([128, S], BF16, tag="kT", name=f"kT{b}")
                psq = psA.tile([128, ST], F32, tag="psA", name=f"pq{b}{sh}{st}")
                psk = psA.tile([128, ST], F32, tag="psA", name=f"pk{b}{sh}{st}")
                csl = slice(st * ST, (st + 1) * ST)
                for c in range(NCHUNK):
                    xc = xts[(b, sh, c)][:, 0, csl]
                    mm(psq, lhsT=wq_sb[:, c, :], rhs=xc,
                       start=(c == 0), stop=(c == NCHUNK - 1),
                       skip_group_check=True)
                    mm(psk, lhsT=wk_sb[:, c, :], rhs=xc,
                       start=(c == 0), stop=(c == NCHUNK - 1),
                       skip_group_check=True)
                g0 = sh * SH + st * ST
                nc.vector.tensor_scalar_add(qTs[b][:, g0 : g0 + ST], psq, bq_sb)
                nc.vector.tensor_scalar_add(kTs[b][:, g0 : g0 + ST], psk, bk_sb)

            # DMA ring order: wq first, then x(b0,sh0) chunks (the critical
            # path), the other consts slotted where their consumers are
            nc.sync.dma_start(out=wq_sb, in_=wq_r[:, :, :])
            for c in range(4):
                dma_x(0, 0, c)
            nc.sync.dma_start(out=bq_sb, in_=bq[:, :])
            nc.sync.dma_start(out=wk_sb, in_=wk_r[:, :, :])
            for c in range(4, NCHUNK):
                dma_x(0, 0, c)
            nc.sync.dma_start(out=bk_sb, in_=bk[:, :])
            nc.sync.dma_start(out=wv_sb, in_=wv_r[:, :, :])
            for c in range(NCHUNK):
                dma_x(0, 1, c)
            nc.sync.dma_start(out=wp_sb, in_=wp[:, :])
            # prefetch b1's x during the otherwise-idle early ring window
            ensure_x(1, 0)
            ensure_x(1, 1)

            # HAM warmup: ~3us of throwaway matmuls during the initial DMA
            # wait so the real A-phase starts at the 2.4GHz clock
            ps_w = psA.tile([128, 512], F32, tag="psA", name="warmps")
            for i in range(7):
                mm(ps_w, lhsT=warm_sb[:, 0:128], rhs=warm_sb,
                   start=(i == 0), stop=(i == 6))

            emit_qk_pair(0, 0, 0)
            emit_qk_pair(0, 0, 1)
            emit_v_group(0, 0)
            emit_qk_pair(0, 1, 0)
            emit_qk_pair(0, 1, 1)
            emit_v_group(0, 1)
            # interleave batch 1's A-phase into batch 0's attention
            items = [
                ("qk", 0, "q"), ("v", 0, None), ("qk", 0, "k"),
                ("qk", 1, "q"), ("v", 1, None), ("qk", 1, "k"),
            ]
            per_qt = -(-len(items) // NQ)
            interleave = {
                qt: items[qt * per_qt : (qt + 1) * per_qt] for qt in range(NQ)
            }
            # Global software pipeline: flat (b, qt, tt) stream; attn@V
            # trails the scores/exp by DELAY slots across q-tile and batch
            # boundaries; the previous q-tile's projection chunks are spread
            # one per slot so their psum evacuations never bunch up.
            DELAY = 5
            seq = [
                (b, qt, tt)
                for b in range(B)
                for qt in range(NQ)
                for tt in range(NTT)
            ]
            a_tiles = {}
            qt_state = {}  # (b, qt) -> out_ps list
            from collections import deque
            pending_proj = deque()

            def boundary(bq, qq, g):
                """av of (bq, qq) just completed: norm now, queue proj
                (first chunk held back 3 slots so it lands after the
                normalization chain has finished)."""
                emit_norm_qt(bq, qq, qt_state.pop((bq, qq)),
                             tail=(bq == B - 1 and qq == NQ - 1))
                for st in range(ST // 128):
                    for eh in range(E // 512):
                        pending_proj.append((g + 3, (bq, qq, st, eh)))
                if bq + 1 < B:
                    for item in interleave.get(qq, []):
                        kind, sh, which = item
                        if kind == "qk":
                            emit_qk_group(bq + 1, sh, which)
                        else:
                            emit_v_group(bq + 1, sh)

            for g, (b, qt, tt) in enumerate(seq):
                emit_sc(b, qt, tt, a_tiles)
                if g >= DELAY:
                    bb, qb, tb = seq[g - DELAY]
                    ops = qt_state.setdefault((bb, qb), [])
                    emit_av(bb, qb, tb, a_tiles, ops)
                    if tb == NTT - 1:
                        boundary(bb, qb, g)
                if pending_proj and pending_proj[0][0] <= g:
                    emit_proj_chunk(*pending_proj.popleft()[1])
            for g in range(len(seq) - DELAY, len(seq)):
                bb, qb, tb = seq[g]
                ops = qt_state.setdefault((bb, qb), [])
                emit_av(bb, qb, tb, a_tiles, ops)
                if tb == NTT - 1:
                    boundary(bb, qb, g)
            k = 0
            while pending_proj:
                emit_proj_chunk(*pending_proj.popleft()[1],
                                evac_act=(k % 2 == 1))
                k += 1

    nc.compile()
    return nc


_NC_CACHE = {}


def _get_nc(B, S):
    key = (B, S)
    if key not in _NC_CACHE:
        _NC_CACHE[key] = build_nc(B, S)
    return _NC_CACHE[key]


def _prearrange_w(w):
    """[E, LF] -> [128, NCHUNK*LF] with out[p, c*LF+n] = w[c*128+p, n]."""
    w3 = w.reshape(NCHUNK, 128, LF)  # [c, p, n]
    return np.ascontiguousarray(w3.transpose(1, 0, 2).reshape(128, NCHUNK * LF))


def make_in_maps(x, w_qkv, b_qkv, w_proj):
    B, S, _ = x.shape
    xT = np.ascontiguousarray(x.reshape(B * S, E).T).astype(NPBF16)
    in_maps = []
    for c in range(NCORES):
        cols = slice(c * LF, (c + 1) * LF)
        in_maps.append(
            {
                "xT": xT,
                "wq": _prearrange_w(
                    w_qkv[:, 0 * E : 1 * E][:, cols] * 16.0
                ).astype(NPBF16),
                "wk": _prearrange_w(
                    w_qkv[:, 1 * E : 2 * E][:, cols] * 16.0
                ).astype(NPBF16),
                "wv": _prearrange_w(
                    w_qkv[:, 2 * E : 3 * E][:, cols] * 16.0
                ).astype(NPBF16),
                "bq": (b_qkv[0 * E : 1 * E][cols] * 16.0)
                .reshape(LF, 1)
                .astype(np.float32),
                "bk": (b_qkv[1 * E : 2 * E][cols] * 16.0)
                .reshape(LF, 1)
                .astype(np.float32),
                "wp": np.ascontiguousarray(w_proj[cols, :] / 16.0).astype(
                    NPBF16
                ),
            }
        )
    return in_maps


def kernel_run(x, w_qkv, b_qkv, w_proj, b_proj, trace=False):
    x = np.asarray(x, dtype=np.float32)
    w_qkv = np.asarray(w_qkv, dtype=np.float32)
    b_qkv = np.asarray(b_qkv, dtype=np.float32)
    w_proj = np.asarray(w_proj, dtype=np.float32)
    b_proj = np.asarray(b_proj, dtype=np.float32)
    B, S, _ = x.shape
    nc = _get_nc(B, S)
    in_maps = make_in_maps(x, w_qkv, b_qkv, w_proj)
    res = run_bass_kernel_spmd(
        nc, in_maps, core_ids=list(range(NCORES)), trace=trace
    )
    y = res.results[0]["y"].astype(np.float64)
    for c in range(1, NCORES):
        y += res.results[c]["y"]
    # v-bias contribution: (b_v @ w_proj) constant row + b_proj
    bv = b_qkv[2 * E : 3 * E]
    y += (bv @ w_proj + b_proj)[None, :]
    return y.astype(np.float32).reshape(B, S, E), res


def kernel(x, w_qkv, b_qkv, w_proj, b_proj):
    y, _ = kernel_run(x, w_qkv, b_qkv, w_proj, b_proj)
    return y
